# revision 17
# baseline (speedup 1.0000x reference)
"""BoundaryAwareViT Trainium2 kernel — nn_BoundaryAwareViT_74500502716591.

kernel(**inputs) takes FULL unsharded inputs (as in setup_inputs) and returns
the FULL [32,1,32,32] f32 output. Internally: data-parallel over batch across
8 NeuronCores (4 images/core, all params replicated), one Bass/Tile program
per core run via run_bass_kernel_spmd.

Device mapping (per core, per image):
  h resident as [tok=128 x 8 tiles, D=256] f32.  LN stats via bn_stats.
  All matmuls contract on partitions; xn is PE-transposed to xnT [D, 1024].
  Criss-cross attention: row groups = 4 grid rows (128 tokens, contiguous);
  col groups r = columns {w : w%8==r} (order o = h*4 + w//8) so the
  col->row denominator permute is 8 small contiguous DMAs.  Softmax is
  unnormalized exp (logits are O(1)); 0/1 masks select row/col neighbors;
  normalization folds into the final per-token scale gamma/den.
  Row-branch apply lands in [tok, D] PSUM directly; col-branch apply lands
  in transposed space, scattered into accT via free-dim APs, transposed back.
  FFN: h1T = w1^T @ xnT (dF on partitions -> gelu bias per-partition),
  h2 accumulated straight into h.
LN scale/shift and biases are folded into weights host-side (exact).
"""

import numpy as np
import ml_dtypes

import concourse.bass as bass
import concourse.tile as tile
from concourse import mybir
from concourse.bass_utils import run_bass_kernel_spmd
from concourse.vector_clock import ScopedClock

F32 = mybir.dt.float32
F32R = mybir.dt.float32r
BF16 = mybir.dt.bfloat16
AF = mybir.ActivationFunctionType
ALU = mybir.AluOpType
AX = mybir.AxisListType

B, IMG, P, D, DEPTH = 32, 512, 16, 256, 8
G = 32
N = 1024
DQ = 32
DF = 1024
N_CORES = 8
IPC = B // N_CORES          # images per core = 4
SCALE = float(1.0 / np.sqrt(np.float32(DQ)))

DT = F32R                   # main-path matmul dtype (f32 bits, 1 cyc/row @ N>=256)
DT_NP = np.float32
ADT = BF16                  # attention-inner dtype (gamma-damped)
ADT_NP = ml_dtypes.bfloat16


class PatchedTileContext(tile.TileContext):
    """Walrus TPB_CTRL codegen rejects >1 sync-wait on the kernel-tail
    Drain; spread the global-clock waits across single-wait NOPs."""

    def _drain_and_barrier(self, tick_clock, wait_clock):
        probe = self.nc.sync.nop(nofuse=True)
        wait_clock.add_sem_waits(
            probe.ins, ScopedClock({None: tick_clock.global_clock})
        )
        si = probe.ins.sync_info
        if si is not None and len(si.on_wait) > 1:
            waits = list(si.on_wait)
            probe.ins.sync_info = mybir.SyncInfo(
                on_wait=[waits[0]], on_update=list(si.on_update)
            )
            for w in waits[1:]:
                n2 = self.nc.sync.nop(nofuse=True)
                n2.ins.sync_info = mybir.SyncInfo(on_wait=[w], on_update=[])
        self.nc.sync.drain()
        self.nc.all_engine_barrier()
        assert self.sems is not None
        popped = self.nc._tile_sem_poison_stack.pop()
        assert popped is self._sem_poison
        self.nc.clear_and_free_semaphores(list(self.sems.allocated().values()))
        self.nc.all_engine_barrier()


def _col_ap(t2d):
    """[p, 1024] -> [p, 32(h), 4(j), 8(r)] view of the token axis."""
    return t2d.rearrange("p (h j r) -> p h j r", h=32, j=4)


def _split_excess_waits(nc, limit=1):
    """Walrus codegen supports very few sync-wait commands per instruction;
    hoist excess waits onto same-engine NOPs inserted just before."""
    ctr = 0
    for fn in nc.m.functions:
        for blk in fn.blocks:
            insts = blk.instructions
            out = []
            changed = False
            for inst in insts:
                si = inst.sync_info
                if si is not None and len(si.on_wait) > limit:
                    waits = list(si.on_wait)
                    for w in waits[:-limit]:
                        nop = mybir.InstNoOp(
                            name=f"waitsplit-{ctr}", engine=inst.engine,
                            ins=[], outs=[])
                        nop.sync_info = mybir.SyncInfo(on_wait=[w], on_update=[])
                        nc.register_instruction(nop, overwrite=True)
                        out.append(nop)
                        ctr += 1
                    inst.sync_info = mybir.SyncInfo(
                        on_wait=waits[-limit:], on_update=list(si.on_update))
                    changed = True
                out.append(inst)
            if changed:
                blk.instructions = out


def build_program(n_layers=DEPTH, flags=()):
    flags = set(flags)
    nc = bass.Bass()

    def param(name, shape, dt):
        return nc.declare_dram_parameter(name, list(shape), dt, isOutput=False)

    xpT_d = param("xpT", [IPC, 128, 2, N], DT)
    wp_d = param("wp", [128, 2, D], DT)
    posT_d = param("posT", [128, 2, N], F32)
    wedge_d = param("wedge", [128, 2, D], DT)
    wq_d = param("wq", [DEPTH, 128, 2, DQ], DT)
    wk_d = param("wk", [DEPTH, 128, 2, DQ], DT)
    wv_d = param("wv", [DEPTH, 128, 2, D], DT)
    w1_d = param("w1", [DEPTH, 128, 2, DF], DT)
    w2_d = param("w2", [DEPTH, 128, 8, D], ADT)
    bq_d = param("bq", [DEPTH, DQ, 1], F32)
    bk_d = param("bk", [DEPTH, DQ, 1], F32)
    b1_d = param("b1", [DEPTH, 128, 8], F32)
    gam_d = param("gam", [DEPTH, 128, 1], F32)
    mrow_d = param("mrow", [128, 128], ADT)
    mcol_d = param("mcol", [128, 128], ADT)
    idf_d = param("idf", [128, 128], F32)
    idr_d = param("idr", [128, 128], F32R)
    idb_d = param("idb", [128, 128], ADT)
    whead_d = param("whead", [128, D], F32)
    # optional (emitted only when the corresponding flag is set)
    if "bv" in flags:
        bv_d = param("bv", [DEPTH, 128, D], F32)
    if "b2" in flags:
        b2_d = param("b2", [DEPTH, 128, D], F32)
    if "bedge" in flags:
        bedge_d = param("bedge", [128, D], F32)
    if "lngb" in flags:
        lng_d = param("lng", [DEPTH, 128, D], F32)
        lnb_d = param("lnb", [DEPTH, 128, D], F32)
    if "bhead" in flags:
        bhead_d = param("bhead", [128, 1], F32)

    out_d = nc.declare_dram_parameter("out", [IPC, N], F32, isOutput=True)

    from contextlib import ExitStack
    with PatchedTileContext(nc) as tc, ExitStack() as es:
        cpool = es.enter_context(tc.tile_pool(name="consts", bufs=1))
        hpool = es.enter_context(tc.tile_pool(name="hres", bufs=1))
        spool = es.enter_context(tc.tile_pool(name="small", bufs=3))
        psA = es.enter_context(tc.tile_pool(name="psA", bufs=2, space="PSUM"))
        psB = es.enter_context(tc.tile_pool(name="psB", bufs=3, space="PSUM"))
        psC = es.enter_context(tc.tile_pool(name="psC", bufs=3, space="PSUM"))

        # ---- constants ----
        ident_f = cpool.tile([128, 128], F32, tag="idf")
        nc.sync.dma_start(out=ident_f, in_=idf_d[:])
        ident_r = cpool.tile([128, 128], F32R, tag="idr")
        nc.sync.dma_start(out=ident_r, in_=idr_d[:])
        ident_b = cpool.tile([128, 128], ADT, tag="idb")
        nc.sync.dma_start(out=ident_b, in_=idb_d[:])
        mrow_t = cpool.tile([128, 128], ADT, tag="mrow")
        nc.sync.dma_start(out=mrow_t, in_=mrow_d[:])
        mcol_t = cpool.tile([128, 128], ADT, tag="mcol")
        nc.sync.dma_start(out=mcol_t, in_=mcol_d[:])
        whead_t = cpool.tile([128, D], F32, tag="whead")
        nc.sync.dma_start(out=whead_t, in_=whead_d[:])
        posT_t = cpool.tile([128, 2, N], F32, tag="posT")
        nc.sync.dma_start(out=posT_t, in_=posT_d[:])
        wp_t = cpool.tile([128, 2, D], DT, tag="wp")
        nc.sync.dma_start(out=wp_t, in_=wp_d[:])
        wedge_t = cpool.tile([128, 2, D], DT, tag="wedge")
        nc.sync.dma_start(out=wedge_t, in_=wedge_d[:])
        eps_t = cpool.tile([128, 1], F32, tag="eps")
        nc.vector.memset(eps_t, 1e-5)
        if "bedge" in flags:
            bedge_t = cpool.tile([128, D], F32, tag="bedge")
            nc.sync.dma_start(out=bedge_t, in_=bedge_d[:])
        if "bhead" in flags:
            bhead_t = cpool.tile([128, 1], F32, tag="bhead")
            nc.sync.dma_start(out=bhead_t, in_=bhead_d[:])

        # ---- residual tiles ----
        h_ts = []
        for i in range(IPC):
            h_ts.append(hpool.tile([128, 8, D], F32, name=f"h{i}", tag=f"h{i}"))

        # ---- embed + edge, per image (pool released before layer pools) ----
        emb_es = ExitStack()
        epool = emb_es.enter_context(tc.tile_pool(name="emb", bufs=2))
        for i in range(IPC):
            h_t = h_ts[i]
            xp_t = epool.tile([128, 2, N], DT, tag="xp")
            nc.sync.dma_start(out=xp_t, in_=xpT_d[i])
            tT = epool.tile([128, 2, N], F32, tag="tT")
            for mc in range(2):
                for ncol in range(2):
                    ps = psA.tile([128, 512], F32, tag="pA")
                    for kc in range(2):
                        nc.tensor.matmul(
                            ps, wp_t[:, kc, mc * 128:(mc + 1) * 128],
                            xp_t[:, kc, ncol * 512:(ncol + 1) * 512],
                            start=(kc == 0), stop=(kc == 1))
                    nc.vector.tensor_add(
                        out=tT[:, mc, ncol * 512:(ncol + 1) * 512],
                        in0=ps, in1=posT_t[:, mc, ncol * 512:(ncol + 1) * 512])
            # h tiles = transpose(tT)
            for t in range(8):
                for kc in range(2):
                    pt = psC.tile([128, 128], F32, tag="pC")
                    nc.tensor.transpose(pt, tT[:, kc, t * 128:(t + 1) * 128], ident_f)
                    nc.vector.tensor_copy(out=h_t[:, t, kc * 128:(kc + 1) * 128], in_=pt)
            # Laplacian in T space
            eT = epool.tile([128, 2, N], DT, tag="eT")
            for kc in range(2):
                nc.vector.tensor_scalar_mul(out=eT[:, kc], in0=tT[:, kc], scalar1=4.0)
                nc.vector.tensor_sub(out=eT[:, kc, 32:], in0=eT[:, kc, 32:],
                                     in1=tT[:, kc, :N - 32])
                nc.vector.tensor_sub(out=eT[:, kc, :N - 32], in0=eT[:, kc, :N - 32],
                                     in1=tT[:, kc, 32:])
                ev = eT[:, kc].rearrange("p (h w) -> p h w", h=32)
                tv = tT[:, kc].rearrange("p (h w) -> p h w", h=32)
                nc.vector.tensor_sub(out=ev[:, :, 1:], in0=ev[:, :, 1:], in1=tv[:, :, :31])
                nc.vector.tensor_sub(out=ev[:, :, :31], in0=ev[:, :, :31], in1=tv[:, :, 1:])
            # edge matmul + tanh + add
            for t in range(8):
                ps = psB.tile([128, D], F32, tag="pB")
                for kc in range(2):
                    nc.tensor.matmul(ps, eT[:, kc, t * 128:(t + 1) * 128],
                                     wedge_t[:, kc], start=(kc == 0), stop=(kc == 1))
                if "bedge" in flags:
                    nc.vector.tensor_add(out=ps, in0=ps, in1=bedge_t)
                etmp = epool.tile([128, D], F32, tag="etmp")
                nc.scalar.activation(out=etmp, in_=ps, func=AF.Tanh)
                nc.vector.tensor_add(out=h_t[:, t], in0=h_t[:, t], in1=etmp)

        emb_es.close()
        wpool = es.enter_context(tc.tile_pool(name="wts", bufs=2))
        zpool = es.enter_context(tc.tile_pool(name="zs", bufs=2))
        xtp = es.enter_context(tc.tile_pool(name="xnt", bufs=2))
        apool = es.enter_context(tc.tile_pool(name="attn", bufs=2))
        gpool = es.enter_context(tc.tile_pool(name="g1t", bufs=1))

        # ---- helpers ----
        def emit_ln_transpose(h_t, ztag, xtag, lgt=None, lbt=None):
            z_t = zpool.tile([128, 8, D], DT, tag=ztag)
            for t in range(8):
                stats = spool.tile([128, 6], F32, tag="bnst")
                nc.vector.bn_stats(out=stats, in_=h_t[:, t])
                mv = spool.tile([128, 2], F32, tag="bnmv")
                nc.vector.bn_aggr(out=mv, in_=stats)
                rstd = spool.tile([128, 1], F32, tag="rstd")
                nc.scalar.activation(out=rstd, in_=mv[:, 1:2], func=AF.Sqrt,
                                     bias=eps_t, scale=1.0)
                nc.vector.reciprocal(out=rstd, in_=rstd)
                nc.vector.tensor_scalar(
                    out=z_t[:, t], in0=h_t[:, t], scalar1=mv[:, 0:1],
                    scalar2=rstd, op0=ALU.subtract, op1=ALU.mult)
            xnT_t = xtp.tile([128, 2, N], DT, tag=xtag)
            for t in range(8):
                for kc in range(2):
                    pt = psC.tile([128, 128], DT, tag="pC")
                    nc.tensor.transpose(pt, z_t[:, t, kc * 128:(kc + 1) * 128],
                                        ident_r if DT == F32R else ident_b)
                    nc.vector.tensor_copy(
                        out=xnT_t[:, kc, t * 128:(t + 1) * 128], in_=pt)
            return z_t, xnT_t

        # ---- transformer layers (unrolled) ----
        for layer in range(n_layers):
            wq_t = wpool.tile([128, 2, DQ], DT, tag="wq")
            nc.sync.dma_start(out=wq_t, in_=wq_d[layer])
            wk_t = wpool.tile([128, 2, DQ], DT, tag="wk")
            nc.sync.dma_start(out=wk_t, in_=wk_d[layer])
            wv_t = wpool.tile([128, 2, D], DT, tag="wv")
            nc.sync.dma_start(out=wv_t, in_=wv_d[layer])
            w1_t = wpool.tile([128, 2, DF], DT, tag="w1")
            nc.sync.dma_start(out=w1_t, in_=w1_d[layer])
            w2_t = wpool.tile([128, 8, D], ADT, tag="w2")
            nc.sync.dma_start(out=w2_t, in_=w2_d[layer])
            bq_t = wpool.tile([DQ, 1], F32, tag="bq")
            nc.sync.dma_start(out=bq_t, in_=bq_d[layer])
            bk_t = wpool.tile([DQ, 1], F32, tag="bk")
            nc.sync.dma_start(out=bk_t, in_=bk_d[layer])
            b1_t = wpool.tile([128, 8], F32, tag="b1")
            nc.sync.dma_start(out=b1_t, in_=b1_d[layer])
            gam_t = wpool.tile([128, 1], F32, tag="gam")
            nc.sync.dma_start(out=gam_t, in_=gam_d[layer])
            if "bv" in flags:
                bv_t = wpool.tile([128, D], F32, tag="bv")
                nc.sync.dma_start(out=bv_t, in_=bv_d[layer])
            if "b2" in flags:
                b2_t = wpool.tile([128, D], F32, tag="b2")
                nc.sync.dma_start(out=b2_t, in_=b2_d[layer])
            if "lngb" in flags:
                lng_t = wpool.tile([128, D], F32, tag="lng")
                nc.sync.dma_start(out=lng_t, in_=lng_d[layer])
                lnb_t = wpool.tile([128, D], F32, tag="lnb")
                nc.sync.dma_start(out=lnb_t, in_=lnb_d[layer])

            for i in range(IPC):
                h_t = h_ts[i]
                # === attention sublayer ===
                z_t, xnT_t = emit_ln_transpose(h_t, "z", "xnT")

                # qT/kT [32, 1024] bf16
                qT_t = apool.tile([DQ, N], ADT, tag="qT")
                kT_t = apool.tile([DQ, N], ADT, tag="kT")
                for (w_t, b_t, o_t) in ((wq_t, bq_t, qT_t), (wk_t, bk_t, kT_t)):
                    for ncol in range(2):
                        ps = psA.tile([DQ, 512], F32, tag="pA")
                        for kc in range(2):
                            nc.tensor.matmul(
                                ps, w_t[:, kc],
                                xnT_t[:, kc, ncol * 512:(ncol + 1) * 512],
                                start=(kc == 0), stop=(kc == 1))
                        nc.vector.tensor_scalar_add(
                            out=o_t[:, ncol * 512:(ncol + 1) * 512],
                            in0=ps, scalar1=b_t)

                # V for 8 row groups then 8 col groups
                V_t = apool.tile([128, 16, D], ADT, tag="V")
                for gi in range(16):
                    if gi < 8:
                        lhs = lambda kc: xnT_t[:, kc, gi * 128:(gi + 1) * 128]
                    else:
                        lhs = lambda kc: _col_ap(xnT_t[:, kc])[:, :, :, gi - 8]
                    ps = psB.tile([128, D], F32, tag="pB")
                    for kc in range(2):
                        nc.tensor.matmul(ps, lhs(kc), wv_t[:, kc],
                                         start=(kc == 0), stop=(kc == 1))
                    if "bv" in flags:
                        nc.vector.tensor_add(out=ps, in0=ps, in1=bv_t)
                    nc.vector.tensor_copy(out=V_t[:, gi], in_=ps)

                # scores -> exp -> mask -> partial sums
                P_t = apool.tile([128, 16, 128], ADT, tag="P")
                rs_t = spool.tile([128, 8], F32, tag="rs")
                cs_t = spool.tile([128, 8], F32, tag="cs")
                for gi in range(16):
                    if gi < 8:
                        qa = qT_t[:, gi * 128:(gi + 1) * 128]
                        ka = kT_t[:, gi * 128:(gi + 1) * 128]
                        m_t = mrow_t
                    else:
                        qa = _col_ap(qT_t)[:, :, :, gi - 8]
                        ka = _col_ap(kT_t)[:, :, :, gi - 8]
                        m_t = mcol_t
                    ps = psC.tile([128, 128], F32, tag="pC")
                    nc.tensor.matmul(ps, qa, ka, start=True, stop=True)
                    e_sb = spool.tile([128, 128], ADT, tag="esb")
                    nc.scalar.activation(out=e_sb, in_=ps, func=AF.Exp, scale=SCALE)
                    nc.vector.tensor_mul(out=P_t[:, gi], in0=e_sb, in1=m_t)
                    dst = rs_t[:, gi:gi + 1] if gi < 8 else cs_t[:, gi - 8:gi - 7]
                    nc.vector.reduce_sum(out=dst, in_=P_t[:, gi], axis=AX.X)

                # denominators: permute cs to row order, combine, invert, scale
                csr_t = spool.tile([128, 8], F32, tag="csr")
                for g in range(8):
                    nc.sync.dma_start(out=csr_t[:, g:g + 1],
                                      in_=cs_t[g * 16:(g + 1) * 16, :])
                sfac_t = spool.tile([128, 8], F32, tag="sfac")
                nc.vector.tensor_add(out=sfac_t, in0=rs_t, in1=csr_t)
                nc.vector.reciprocal(out=sfac_t, in_=sfac_t)
                nc.vector.tensor_scalar_mul(out=sfac_t, in0=sfac_t, scalar1=gam_t)

                # col branch applies -> accT
                accT_t = apool.tile([128, 2, N], F32, tag="accT")
                for r in range(8):
                    ptp = psC.tile([128, 128], ADT, tag="pC")
                    nc.tensor.transpose(ptp, P_t[:, 8 + r], ident_b)
                    pt_sb = spool.tile([128, 128], ADT, tag="ptsb")
                    nc.vector.tensor_copy(out=pt_sb, in_=ptp)
                    for kc in range(2):
                        cps = psC.tile([128, 128], F32, tag="pC")
                        nc.tensor.matmul(cps, V_t[:, 8 + r, kc * 128:(kc + 1) * 128],
                                         pt_sb, start=True, stop=True)
                        dst = _col_ap(accT_t[:, kc])[:, :, :, r]
                        nc.vector.tensor_copy(
                            out=dst, in_=cps.rearrange("p (h j) -> p h j", h=32))

                # row branch applies + h update
                for g in range(8):
                    ptp = psC.tile([128, 128], ADT, tag="pC")
                    nc.tensor.transpose(ptp, P_t[:, g], ident_b)
                    pt_sb = spool.tile([128, 128], ADT, tag="ptsb")
                    nc.vector.tensor_copy(out=pt_sb, in_=ptp)
                    rps = psB.tile([128, D], F32, tag="pB")
                    nc.tensor.matmul(rps, pt_sb, V_t[:, g], start=True, stop=True)
                    aps = psB.tile([128, D], F32, tag="pB")
                    for kc in range(2):
                        nc.tensor.transpose(aps[:, kc * 128:(kc + 1) * 128],
                                            accT_t[:, kc, g * 128:(g + 1) * 128],
                                            ident_f)
                    # h += sfac*rps ; h += sfac*aps  (one PSUM input per DVE op)
                    nc.vector.scalar_tensor_tensor(
                        out=h_t[:, g], in0=rps, scalar=sfac_t[:, g:g + 1],
                        in1=h_t[:, g], op0=ALU.mult, op1=ALU.add)
                    nc.vector.scalar_tensor_tensor(
                        out=h_t[:, g], in0=aps, scalar=sfac_t[:, g:g + 1],
                        in1=h_t[:, g], op0=ALU.mult, op1=ALU.add)
                    if "lngb" in flags:
                        xr = spool.tile([128, D], F32, tag="xr")
                        nc.vector.tensor_mul(out=xr, in0=z_t[:, g], in1=lng_t)
                        nc.vector.tensor_add(out=xr, in0=xr, in1=lnb_t)
                        nc.vector.tensor_add(out=h_t[:, g], in0=h_t[:, g], in1=xr)
                    else:
                        nc.vector.tensor_add(out=h_t[:, g], in0=h_t[:, g],
                                             in1=z_t[:, g])

                # === FFN sublayer ===
                _, xn2T_t = emit_ln_transpose(h_t, "z", "xnT")
                g1_t = gpool.tile([128, 8, DF], ADT, tag="g1")
                for mc in range(8):
                    for ncol in range(2):
                        ps = psA.tile([128, 512], F32, tag="pA")
                        for kc in range(2):
                            nc.tensor.matmul(
                                ps, w1_t[:, kc, mc * 128:(mc + 1) * 128],
                                xn2T_t[:, kc, ncol * 512:(ncol + 1) * 512],
                                start=(kc == 0), stop=(kc == 1))
                        nc.scalar.activation(
                            out=g1_t[:, mc, ncol * 512:(ncol + 1) * 512], in_=ps,
                            func=AF.Gelu, bias=b1_t[:, mc:mc + 1], scale=1.0)
                for t in range(8):
                    ps = psB.tile([128, D], F32, tag="pB")
                    for kc in range(8):
                        nc.tensor.matmul(ps, g1_t[:, kc, t * 128:(t + 1) * 128],
                                         w2_t[:, kc], start=(kc == 0), stop=(kc == 7))
                    if "b2" in flags:
                        nc.vector.tensor_add(out=ps, in0=ps, in1=b2_t)
                    nc.vector.tensor_add(out=h_t[:, t], in0=h_t[:, t], in1=ps)

        # ---- head ----
        for i in range(IPC):
            o_t = spool.tile([128, 8], F32, tag="oimg")
            for t in range(8):
                htmp = spool.tile([128, D], F32, tag="htmp")
                nc.vector.tensor_mul(out=htmp, in0=h_ts[i][:, t], in1=whead_t)
                nc.vector.reduce_sum(out=o_t[:, t:t + 1], in_=htmp, axis=AX.X)
            if "bhead" in flags:
                nc.vector.tensor_scalar_add(out=o_t, in0=o_t, scalar1=bhead_t)
            nc.sync.dma_start(
                out=out_d[i].rearrange("(t p) -> p t", p=128), in_=o_t)

    _split_excess_waits(nc)
    return nc


# ---------------- host side ----------------

_PROGRAM_CACHE = {}


def _get_program(n_layers, flags):
    key = (n_layers, tuple(sorted(flags)))
    if key not in _PROGRAM_CACHE:
        _PROGRAM_CACHE[key] = build_program(n_layers, flags)
    return _PROGRAM_CACHE[key]


def _rep(v, n=128):
    return np.broadcast_to(np.asarray(v, np.float32)[None, :], (n, len(v))).copy()


def prep_inputs(inputs, n_layers=DEPTH):
    """Host-side layout prep. Returns (core_maps, flags)."""
    f32 = np.float32
    x = np.asarray(inputs["x"], f32)
    w_patch = np.asarray(inputs["w_patch"], f32)
    b_patch = np.asarray(inputs["b_patch"], f32)
    pos = np.asarray(inputs["pos"], f32)
    w_edge = np.asarray(inputs["w_edge"], f32)
    b_edge = np.asarray(inputs["b_edge"], f32)
    ln_g = np.asarray(inputs["ln_g"], f32)
    ln_b = np.asarray(inputs["ln_b"], f32)
    wq = np.asarray(inputs["wq"], f32)
    bq = np.asarray(inputs["bq"], f32)
    wk = np.asarray(inputs["wk"], f32)
    bk = np.asarray(inputs["bk"], f32)
    wv = np.asarray(inputs["wv"], f32)
    bv = np.asarray(inputs["bv"], f32)
    gamma = np.asarray(inputs["gamma"], f32)
    w1 = np.asarray(inputs["w1"], f32)
    b1 = np.asarray(inputs["b1"], f32)
    w2 = np.asarray(inputs["w2"], f32)
    b2 = np.asarray(inputs["b2"], f32)
    w_head = np.asarray(inputs["w_head"], f32)
    b_head = np.asarray(inputs["b_head"], f32)

    flags = set()
    if np.any(bv) or np.any(ln_b @ wv):
        flags.add("bv")
    if np.any(b2):
        flags.add("b2")
    if np.any(b_edge):
        flags.add("bedge")
    if np.any(b_head):
        flags.add("bhead")
    if np.any(ln_g != 1.0) or np.any(ln_b):
        flags.add("lngb")

    # patch extraction: xp[b, tok, pp], then transpose -> [b, pp, tok]
    xp = (x.reshape(B, G, P, G, P).transpose(0, 1, 3, 2, 4).reshape(B, N, P * P))
    xpT = np.ascontiguousarray(xp.transpose(0, 2, 1))          # [B, 256, 1024]
    xpT_h = np.ascontiguousarray(
        xpT.reshape(B, 2, 128, N).transpose(0, 2, 1, 3)).astype(DT_NP)

    wp = w_patch.reshape(D, P * P).T                            # [256, 256]
    wp_h = np.ascontiguousarray(wp.reshape(2, 128, D).transpose(1, 0, 2)).astype(DT_NP)

    posb = pos[0] + b_patch[None, :]                            # [1024, 256]
    posT = posb.T                                               # [256, 1024]
    posT_h = np.ascontiguousarray(posT.reshape(2, 128, N).transpose(1, 0, 2))

    wedge_h = np.ascontiguousarray(
        w_edge.reshape(2, 128, D).transpose(1, 0, 2)).astype(DT_NP)

    def fold_in(w_l, g):                                       # [Din, M] * g[Din]
        return g[:, None] * w_l

    wq_h = np.empty((DEPTH, 128, 2, DQ), DT_NP)
    wk_h = np.empty((DEPTH, 128, 2, DQ), DT_NP)
    wv_h = np.empty((DEPTH, 128, 2, D), DT_NP)
    w1_h = np.empty((DEPTH, 128, 2, DF), DT_NP)
    w2_h = np.empty((DEPTH, 128, 8, D), ADT_NP)
    bq_h = np.empty((DEPTH, DQ, 1), f32)
    bk_h = np.empty((DEPTH, DQ, 1), f32)
    b1_h = np.empty((DEPTH, 128, 8), f32)
    bv_h = np.empty((DEPTH, 128, D), f32)
    b2_h = np.empty((DEPTH, 128, D), f32)
    for l in range(DEPTH):
        g, bb = ln_g[l], ln_b[l]
        wqf = fold_in(wq[l], g) * SCALE
        bqf = (bq[l] + bb @ wq[l]) * SCALE
        wkf = fold_in(wk[l], g)
        bkf = bk[l] + bb @ wk[l]
        wvf = fold_in(wv[l], g)
        bvf = bv[l] + bb @ wv[l]
        w1f = fold_in(w1[l], g)
        b1f = b1[l] + bb @ w1[l]
        wq_h[l] = wqf.reshape(2, 128, DQ).transpose(1, 0, 2)
        wk_h[l] = wkf.reshape(2, 128, DQ).transpose(1, 0, 2)
        wv_h[l] = wvf.reshape(2, 128, D).transpose(1, 0, 2)
        w1_h[l] = w1f.reshape(2, 128, DF).transpose(1, 0, 2)
        w2_h[l] = w2[l].reshape(8, 128, D).transpose(1, 0, 2)
        bq_h[l] = bqf[:, None]
        bk_h[l] = bkf[:, None]
        b1_h[l] = b1f.reshape(8, 128).T
        bv_h[l] = np.broadcast_to(bvf[None, :], (128, D))
        b2_h[l] = np.broadcast_to(b2[l][None, :], (128, D))
    gam_h = np.broadcast_to(gamma[:, None, None], (DEPTH, 128, 1)).copy()

    ii = np.arange(128)
    mrow = (ii[:, None] // 32 == ii[None, :] // 32).astype(ADT_NP)
    mcol = ((ii[:, None] % 4 == ii[None, :] % 4)
            & (ii[:, None] // 4 != ii[None, :] // 4)).astype(ADT_NP)
    ident = np.eye(128, dtype=f32)

    whead_h = np.broadcast_to(w_head[:, 0][None, :], (128, D)).copy()

    shared = dict(
        wp=wp_h, posT=posT_h, wedge=wedge_h, wq=wq_h, wk=wk_h, wv=wv_h,
        w1=w1_h, w2=w2_h, bq=bq_h, bk=bk_h, b1=b1_h, gam=gam_h,
        mrow=mrow, mcol=mcol, idf=ident, idr=ident.astype(DT_NP),
        idb=ident.astype(ADT_NP), whead=whead_h,
    )
    if "bv" in flags:
        shared["bv"] = bv_h
    if "b2" in flags:
        shared["b2"] = b2_h
    if "bedge" in flags:
        shared["bedge"] = np.broadcast_to(b_edge[None, :], (128, D)).copy()
    if "bhead" in flags:
        shared["bhead"] = np.full((128, 1), b_head[0], f32)
    if "lngb" in flags:
        shared["lng"] = np.broadcast_to(ln_g[:, None, :], (DEPTH, 128, D)).copy()
        shared["lnb"] = np.broadcast_to(ln_b[:, None, :], (DEPTH, 128, D)).copy()

    core_maps = []
    for c in range(N_CORES):
        m = dict(shared)
        m["xpT"] = xpT_h[c * IPC:(c + 1) * IPC]
        core_maps.append(m)
    return core_maps, flags


def run_on_device(inputs, n_layers=DEPTH, **run_kwargs):
    core_maps, flags = prep_inputs(inputs, n_layers)
    nc = _get_program(n_layers, flags)
    res = run_bass_kernel_spmd(nc, core_maps, list(range(N_CORES)), **run_kwargs)
    outs = np.concatenate([res.results[c]["out"] for c in range(N_CORES)], axis=0)
    return outs.reshape(B, 1, G, G).astype(np.float32), res


def kernel(**inputs) -> np.ndarray:
    out, _ = run_on_device(inputs)
    return out
